# revision 1
# baseline (speedup 1.0000x reference)
"""Trainium2 Bass kernel for nn_DenseAttentionOneHead — collective-free variant.

out_b = X_b (W^T (X_b^T X_b)).  The D (=1024) output columns split into 8
independent 256-column slices (4 per batch): per core,
  S_sl = X_b^T X_b[:, sl]     ([1024, 256], full-batch contraction)
  M_sl = W^T S_sl             ([1024, 256])
  out[:, sl] = X_b M_sl       ([4096, 256])
No inter-core communication at all; each core streams the full batch in fp16
(lhsT from X rows, rhs from the X column-slice), holds X_b^T for the final
matmul's stationary blocks, and writes its fp32 column slice.
"""

import numpy as np

import concourse.mybir as mybir
import concourse.tile as tile
from concourse import bacc
from concourse.bass_utils import run_bass_kernel_spmd

F32 = mybir.dt.float32
F16 = mybir.dt.float16
P = 128
D = 1024
B = 2
N = 4096
NCORES = 8
GROUP = 4            # cores per batch
SL = D // GROUP      # 256-column slice per core
NO = D // P          # 8
NCH = N // P         # 32 row chunks of the full batch
NT = N // P          # 32 output row tiles

_compiled = None


def _build():
    nc = bacc.Bacc(None, target_bir_lowering=False, debug=False, num_devices=NCORES)

    # xf arrives column-rotated per core (its 256 target columns first) and
    # wf row-rotated identically, so the same program computes every slice.
    xf = nc.dram_tensor("xf", [N, D], F16, kind="ExternalInput")
    xt = nc.dram_tensor("xt", [D, N], F16, kind="ExternalInput")
    wf = nc.dram_tensor("wf", [D, D], F16, kind="ExternalInput")
    o_out = nc.dram_tensor("o_out", [N, SL], F32, kind="ExternalOutput")

    with tile.TileContext(nc) as tc:
        with (
            tc.tile_pool(name="big", bufs=1) as big,
            tc.tile_pool(name="xin", bufs=10) as xin,
            tc.tile_pool(name="stage", bufs=4) as stage,
            tc.tile_pool(name="psum", bufs=8, space="PSUM") as psum,
        ):
            Xt = big.tile([P, NO, N], F16, tag="Xt")       # X^T [a, n], 8MB
            Wsb = big.tile([P, NO, D], F16, tag="W")       # W   [e, a], 2MB
            Ssl = big.tile([P, NO, SL], F16, tag="Ssl")    # S_sl [e, d_sl]
            Msl = big.tile([P, NO, SL], F16, tag="Msl")    # M_sl [a, d_sl]

            # W on gpsimd; X^T queued on sync *after* the row stream below so
            # it self-throttles behind the S-phase stream (needed only by out)
            for ch in range(NO):
                nc.gpsimd.dma_start(Wsb[:, ch, :], wf[ch * P : (ch + 1) * P, :])

            # ---- S_sl = X^T X[:, sl], chunk-outer over the full batch
            accs = [
                psum.tile([P, 512], F32, tag="acc", name=f"sacc_{et}")[:, :SL]
                for et in range(NO)
            ]
            for ch in range(NCH):
                xc = xin.tile([P, D], F16, tag="xc")
                eng = nc.sync if ch % 2 == 0 else nc.scalar
                eng.dma_start(xc[:], xf[ch * P : (ch + 1) * P, :])
                for et in range(NO):
                    nc.tensor.matmul(
                        accs[et][:],
                        xc[:, et * P : (et + 1) * P],
                        xc[:, :SL],
                        start=(ch == 0),
                        stop=(ch == NCH - 1),
                    )
            for ch in range(NO):
                nc.sync.dma_start(Xt[:, ch, :], xt[ch * P : (ch + 1) * P, :])
            for et in range(NO):
                nc.vector.tensor_copy(Ssl[:, et, :], accs[et][:])

            # ---- M_sl = W^T S_sl : lhsT = W[e_ch, a_tile], rhs = S_sl[e_ch, :]
            for at in range(NO):
                acc = psum.tile([P, 512], F32, tag="acc", name=f"macc_{at}")[:, :SL]
                for ch in range(NO):
                    nc.tensor.matmul(
                        acc[:],
                        Wsb[:, ch, at * P : (at + 1) * P],
                        Ssl[:, ch, :],
                        start=(ch == 0),
                        stop=(ch == NO - 1),
                    )
                nc.vector.tensor_copy(Msl[:, at, :], acc[:])

            # ---- out[:, sl] = X M_sl : lhsT = X^T blocks, rhs = M_sl
            for nt in range(NT):
                acc = psum.tile([P, 512], F32, tag="acc", name=f"oacc_{nt}")[:, :SL]
                for ch in range(NO):
                    nc.tensor.matmul(
                        acc[:],
                        Xt[:, ch, nt * P : (nt + 1) * P],
                        Msl[:, ch, :],
                        start=(ch == 0),
                        stop=(ch == NO - 1),
                    )
                ot = stage.tile([P, SL], F32, tag="ot")
                nc.vector.tensor_copy(ot[:], acc[:])
                nc.scalar.dma_start(o_out[nt * P : (nt + 1) * P, :], ot[:])

    nc.finalize()
    return nc


def _get_compiled():
    global _compiled
    if _compiled is None:
        _compiled = _build()
    return _compiled


def kernel(hidden_states, queries, _trace=False, _trace_cores=None):
    x = np.ascontiguousarray(np.asarray(hidden_states, dtype=np.float32))
    w = np.ascontiguousarray(np.asarray(queries, dtype=np.float32))
    assert x.shape == (B, N, D) and w.shape == (D, D)

    nc = _get_compiled()
    w16 = w.astype(np.float16)
    x16 = [x[b].astype(np.float16) for b in range(B)]
    xt16 = [np.ascontiguousarray(x16[b].T) for b in range(B)]
    in_maps = []
    for c in range(NCORES):
        b, s = c // GROUP, c % GROUP
        in_maps.append(
            {
                "xf": np.ascontiguousarray(np.roll(x16[b], -s * SL, axis=1)),
                "xt": xt16[b],
                "wf": np.ascontiguousarray(np.roll(w16, -s * SL, axis=0)),
            }
        )

    res = run_bass_kernel_spmd(
        nc,
        in_maps,
        core_ids=list(range(NCORES)),
        trace=_trace,
        trace_cores=_trace_cores,
    )

    out = np.empty((B, N, D), dtype=np.float32)
    for c in range(NCORES):
        b, s = c // GROUP, c % GROUP
        out[b, :, s * SL : (s + 1) * SL] = res.results[c]["o_out"]

    if _trace:
        kernel.last_result = res
    return out

